# revision 1
# baseline (speedup 1.0000x reference)
"""FP8Linear (blockwise fp8 quant-dequant + matmul) Trainium2 Bass kernel.

Reference semantics (per 128-contiguous-element block, flattened):
    s = max|block| / 448 ; q = fp8_e4m3fn(block / s) ; deq = q * s
    out = x_deq @ w_deq.T

Strategy:
  - Data-parallel over rows of x across 8 NeuronCores (16384/8 = 2048 rows
    per core); weight replicated.
  - Quantization blocks (128 contiguous) lie along the contraction dim K, so
    per-row-tile scales are computed with a free-dim segmented abs-max.
  - TRN fp8e4 has max +-240 (not e4m3fn's 448).  Quantizing v/4 on the TRN
    grid and dequantizing with 4*s reproduces e4m3fn rounding exactly for all
    |v| >= 2^-4 (same significand bit patterns, RNE ties identical); smaller
    values land on a 2x coarser subnormal grid with negligible (<1e-6 L2)
    effect.  s4 = amax*(4/448) wobbles <=1ulp vs jax's division - well under
    the bf16 operand rounding that dominates the (small) overall error.
  - Dequantized operands are bf16; matmul accumulates fp32 in PSUM.
  - Both matmul operands need K on partitions: dequantized tiles are
    transposed SBUF->SBUF with the DMA xbar (bf16 2-byte path).
"""

import sys

for _p in ("/opt/trn_rl_repo",):
    if _p not in sys.path:
        sys.path.insert(0, _p)

from contextlib import ExitStack

import numpy as np

import concourse.bass as bass  # noqa: F401  (registers engines)
import concourse.tile as tile
from concourse import bacc, mybir
from concourse.bass_utils import run_bass_kernel_spmd

P = 128
N_CORES = 8
SKIP_MM = False   # probe: drop matmuls+evicts to isolate the DMA/vector pipeline
B, T, D, OUT = 4, 4096, 2048, 2048
M_FULL = B * T                # 16384
M_CORE = M_FULL // N_CORES    # 2048 rows of x per core


def build(nc, M, K, N, FREE=512, reps=1):
    """Emit the per-core kernel: out[M,N] = qd(x)[M,K] @ qd(w)[N,K].T

    reps>1 repeats the whole computation back-to-back in one program
    (for steady-state timing via the (T_R - T_1)/(R-1) delta method).
    """
    f32 = mybir.dt.float32
    bf16 = mybir.dt.bfloat16
    fp8 = mybir.dt.float8e4

    KB = K // P     # k-blocks (quant blocks == matmul k-tiles)
    NJ = N // FREE  # psum column chunks

    x_d = nc.dram_tensor("x", [M, K], f32, kind="ExternalInput").ap()
    w_d = nc.dram_tensor("w", [N, K], f32, kind="ExternalInput").ap()
    o_d = nc.dram_tensor("out", [M, N], f32, kind="ExternalOutput").ap()

    with tile.TileContext(nc) as tc, ExitStack() as ctx:
        raw = ctx.enter_context(tc.tile_pool(name="raw", bufs=3))
        qp = ctx.enter_context(tc.tile_pool(name="qp", bufs=4))
        dqp = ctx.enter_context(tc.tile_pool(name="dqp", bufs=4))
        scl = ctx.enter_context(tc.tile_pool(name="scl", bufs=3))
        wTp = ctx.enter_context(tc.tile_pool(name="wTp", bufs=1))
        xTp = ctx.enter_context(tc.tile_pool(name="xTp", bufs=5))
        outp = ctx.enter_context(tc.tile_pool(name="outp", bufs=2))
        psum = ctx.enter_context(tc.tile_pool(name="psum", bufs=2, space="PSUM"))


        x_d3 = x_d.rearrange("(t p) k -> t p k", p=P)
        w_d3 = w_d.rearrange("(t p) k -> t p k", p=P)

        def quant_dequant_pair(src_ap, deq_engine, quant_engine=None, load_engine=None):
            load_engine = load_engine or nc.sync
            """DMA a [2, 128, K] f32 pair of row-tiles (one 2 MB transfer),
            blockwise quant-dequant both -> two [P, KB, P] bf16 tiles.

            Engine split: abs-max reduce + quantize on DVE (single broadcast
            ops); dequantize on DVE (single op) or ACT (16 per-block scaled
            copies).  GPSIMD tensor ops crash the exec unit on HW - avoid."""
            rawt = raw.tile([P, 2, K], f32, tag="raw")
            load_engine.dma_start(rawt[:], src_ap.rearrange("t p k -> p t k"))
            r4 = rawt[:].rearrange("p t (b q) -> p t b q", q=P)

            amax = scl.tile([P, 2, KB], f32, tag="amax")
            nc.vector.tensor_reduce(
                amax[:], r4, axis=mybir.AxisListType.X,
                op=mybir.AluOpType.max, apply_absolute_value=True,
            )
            # s4 ~= (amax/448)*4 (DVE ISA has no divide; the <=1ulp wobble
            # vs jax's division shifts the dequant grid by ~1e-7 relative,
            # well under the bf16 operand rounding that dominates error).
            s4 = scl.tile([P, 2, KB], f32, tag="s4")
            nc.vector.tensor_scalar(
                s4[:], amax[:], 4.0 / 448.0, None, op0=mybir.AluOpType.mult,
            )
            rinv4 = scl.tile([P, 2, KB], f32, tag="rinv4")
            nc.vector.reciprocal(rinv4[:], s4[:])

            outs = []
            for t in range(2):
                # Quantize: q = fp8e4(x * (1/s4)). DVE does it in one
                # broadcast op; ACT uses 16 per-block scaled copies.
                qt = qp.tile([P, KB, P], fp8, tag="qt")
                if quant_engine is nc.scalar:
                    for b_ in range(KB):
                        nc.scalar.mul(
                            qt[:, b_], r4[:, t, b_], rinv4[:, t, b_ : b_ + 1]
                        )
                else:
                    nc.vector.tensor_tensor(
                        qt[:], r4[:, t],
                        rinv4[:, t, :, None].broadcast_to((P, KB, P)),
                        op=mybir.AluOpType.mult,
                    )
                # Dequantize: dq = q * s4 (bf16 out).
                dqt = dqp.tile([P, KB, P], bf16, tag="dqt")
                if deq_engine is nc.scalar:
                    for b_ in range(KB):
                        nc.scalar.mul(
                            dqt[:, b_], qt[:, b_], s4[:, t, b_ : b_ + 1]
                        )
                else:
                    deq_engine.tensor_tensor(
                        dqt[:], qt[:],
                        s4[:, t, :, None].broadcast_to((P, KB, P)),
                        op=mybir.AluOpType.mult,
                    )
                outs.append(dqt)
            return outs

        def one_pass(rep):
            # Persistent K-major dequantized weight cache, split by N chunk
            # so matmuls only depend on the 4 transposes feeding their chunk.
            # Layout [P, 4wt, KB, 128]: each w row-tile's transpose lands in
            # a contiguous [P, KB, 128] block (fast xbar path, same as xT);
            # the matmul rhs reads a strided [P, 4, 128] = 512-wide AP.
            wT = [
                wTp.tile([P, FREE // P, KB, P], bf16, tag=f"wT{j}",
                         name=f"wT{j}_{rep}")
                for j in range(NJ)
            ]
            o_d3 = o_d.rearrange("(t p) n -> t p n", p=P)
            NW = N // P // 2
            NX = M // P // 2
            # x-pairs whose prep is interleaved into the w-phase (their
            # matmuls are emitted after ALL wT writes: Tile tracks deps in
            # program order, so a wT read emitted before its write would
            # silently read stale data).
            NPRE = min(NX, 2)

            def w_pair(wp):
                # w-quant on ACT to unclog the serial DVE chain in the head;
                # reduce + deq stay on DVE.
                dqts = quant_dequant_pair(w_d3[2 * wp : 2 * wp + 2], nc.vector, load_engine=nc.scalar)
                for t in range(2):
                    wt = 2 * wp + t
                    j, jj = wt // (FREE // P), wt % (FREE // P)
                    nc.sync.dma_start_transpose(wT[j][:, jj], dqts[t][:])

            def x_prep(mp):
                dqts = quant_dequant_pair(x_d3[2 * mp : 2 * mp + 2], nc.scalar)
                pair = []
                for t in range(2):
                    xT = xTp.tile([P, KB, P], bf16, tag="xT", name=f"xT_{rep}_{mp}_{t}")
                    nc.sync.dma_start_transpose(xT[:], dqts[t][:])
                    pair.append(xT)
                return pair

            def x_mm(mp, xTs, j_outer):
                if SKIP_MM:
                    return
                outt = outp.tile([P, 2, N], f32, tag="outt")
                for t in range(2):
                    mt = 2 * mp + t
                    pst = [
                        psum.tile([P, FREE], f32, tag=f"ps{j}",
                                  name=f"ps{j}_{rep}_{mt}")
                        for j in range(NJ)
                    ]
                    if j_outer:
                        # j-outer: chunk j's matmuls wait only on wT[j], so
                        # the PE starts on wT[0] while later chunks finish.
                        for j in range(NJ):
                            for kb in range(KB):
                                nc.tensor.matmul(
                                    pst[j][:], lhsT=xTs[t][:, kb, :],
                                    rhs=wT[j][:, :, kb, :],
                                    start=(kb == 0), stop=(kb == KB - 1),
                                )
                    else:
                        for kb in range(KB):
                            for j in range(NJ):
                                nc.tensor.matmul(
                                    pst[j][:], lhsT=xTs[t][:, kb, :],
                                    rhs=wT[j][:, :, kb, :],
                                    start=(kb == 0), stop=(kb == KB - 1),
                                )
                    for j in range(NJ):
                        nc.scalar.copy(
                            outt[:, t, j * FREE : (j + 1) * FREE], pst[j][:]
                        )
                nc.scalar.dma_start(
                    o_d3[2 * mp : 2 * mp + 2].rearrange("t p n -> p t n"),
                    outt[:],
                )

            # Emission: (2 w-pairs, 1 x-prep) chunks keep the DVE feeding the
            # PE's early tiles while wT chunks land just in time; all wT
            # writes precede every matmul in program order.
            pre = {}
            wq = list(range(NW))
            for i in range(NPRE):
                for _ in range(2):
                    if wq:
                        w_pair(wq.pop(0))
                pre[i] = x_prep(i)
            while wq:
                w_pair(wq.pop(0))
            for mp in range(NPRE):
                x_mm(mp, pre.pop(mp), j_outer=True)
            for mp in range(NPRE, NX):
                x_mm(mp, x_prep(mp), j_outer=False)

        if reps == 1:
            one_pass(0)
        else:
            # Hardware loop: body emitted once, executed `reps` times on
            # device (for steady-state timing). Back-edge costs ~2-4 us.
            with tc.For_i(0, reps, 1):
                one_pass(0)

    return nc


_NCS = {}


def _get_nc(reps=1):
    if reps not in _NCS:
        nc = bacc.Bacc(
            "TRN2", target_bir_lowering=False, debug=False,
            enable_asserts=False, num_devices=N_CORES,
        )
        build(nc, M_CORE, D, OUT, reps=reps)
        nc.compile()
        _NCS[reps] = nc
    return _NCS[reps]


def _in_maps(x, weight):
    x2 = np.ascontiguousarray(
        np.asarray(x, dtype=np.float32).reshape(M_FULL, D)
    )
    w = np.ascontiguousarray(np.asarray(weight, dtype=np.float32))
    return [
        {"x": x2[c * M_CORE : (c + 1) * M_CORE], "w": w}
        for c in range(N_CORES)
    ]


def kernel(x, weight):
    nc = _get_nc()
    res = run_bass_kernel_spmd(nc, _in_maps(x, weight), core_ids=list(range(N_CORES)))
    out = np.concatenate(
        [np.asarray(res.results[c]["out"], dtype=np.float32) for c in range(N_CORES)],
        axis=0,
    )
    return out.reshape(B, T, OUT)


class _Runner:
    """Reusable jitted single-NEFF-execution runner (device-resident inputs)."""

    def __init__(self, nc):
        import jax
        from jax.experimental.shard_map import shard_map
        from jax.sharding import Mesh, NamedSharding, PartitionSpec

        from concourse import bass2jax

        bass2jax.install_neuronx_cc_hook()
        self.jax = jax
        self.nc = nc

        in_names, out_names, out_avals = [], [], []
        for alloc in nc.m.functions[0].allocations:
            if not isinstance(alloc, mybir.MemoryLocationSet):
                continue
            name = alloc.memorylocations[0].name
            if alloc.kind == "ExternalInput":
                in_names.append(name)
            elif alloc.kind == "ExternalOutput":
                out_names.append(name)
                out_avals.append(
                    jax.core.ShapedArray(
                        tuple(alloc.tensor_shape), mybir.dt.np(alloc.dtype)
                    )
                )
        partition_name = (
            nc.partition_id_tensor.name if nc.partition_id_tensor else None
        )
        in_names = [n for n in in_names if n != partition_name]
        assert in_names == ["x", "w"] and out_names == ["out"]
        all_names = in_names + out_names
        if partition_name is not None:
            all_names.append(partition_name)
        all_names = tuple(all_names)

        def _exec(xa, wa, za):
            operands = [xa, wa, za]
            if partition_name is not None:
                operands.append(bass2jax.partition_id_tensor())
            outs = bass2jax._bass_exec_p.bind(
                *operands,
                out_avals=tuple(out_avals),
                in_names=all_names,
                out_names=tuple(out_names),
                lowering_input_output_aliases=(),
                sim_require_finite=True,
                sim_require_nnan=True,
                nc=nc,
            )
            return (outs[0],)

        devices = jax.devices()[:N_CORES]
        self.mesh = Mesh(np.asarray(devices), ("core",))
        self.sharding = NamedSharding(self.mesh, PartitionSpec("core"))
        self.fn = jax.jit(
            shard_map(
                _exec, mesh=self.mesh,
                in_specs=(PartitionSpec("core"),) * 3,
                out_specs=(PartitionSpec("core"),),
                check_rep=False,
            ),
            donate_argnums=(2,),
            keep_unused=True,
        )
        self.zfn = jax.jit(
            lambda: jax.numpy.zeros((M_FULL, OUT), np.float32),
            out_shardings=self.sharding,
        )

    def put_inputs(self, x, weight):
        maps = _in_maps(x, weight)
        cx = np.concatenate([m["x"] for m in maps], axis=0)
        cw = np.concatenate([m["w"] for m in maps], axis=0)
        self.dx = self.jax.device_put(cx, self.sharding)
        self.dw = self.jax.device_put(cw, self.sharding)

    def run(self):
        out = self.fn(self.dx, self.dw, self.zfn())[0]
        self.jax.block_until_ready(out)
        return out

    def time_min(self, n=5):
        import time

        best = float("inf")
        for _ in range(n):
            z = self.zfn()
            self.jax.block_until_ready(z)
            t0 = time.perf_counter()
            out = self.fn(self.dx, self.dw, z)[0]
            self.jax.block_until_ready(out)
            best = min(best, time.perf_counter() - t0)
        return best


def kernel_timed(x, weight, reps=64, n=6):
    """Returns (out, est_exec_ns).

    No NTFF profiling is available under this axon container.  Estimate the
    per-execution device time with the delta method: compile the same kernel
    with its body repeated `reps` times in one NEFF and report
    (T_reps - T_1) / (reps - 1), cancelling fixed dispatch overhead.
    """
    r1 = _Runner(_get_nc(1))
    r1.put_inputs(x, weight)
    out = np.asarray(r1.run(), dtype=np.float32)
    t1 = r1.time_min(n)

    rR = _Runner(_get_nc(reps))
    rR.put_inputs(x, weight)
    outR = np.asarray(rR.run(), dtype=np.float32)
    tR = rR.time_min(n)
    assert np.array_equal(out, outR), "reps variant disagrees"

    est_ns = (tR - t1) / (reps - 1) * 1e9
    print(f"[timing] T1={t1*1e3:.3f} ms  T{reps}={tR*1e3:.3f} ms  "
          f"delta/iter={est_ns/1e3:.1f} us")
    return out.reshape(B, T, OUT), int(est_ns)

